# revision 1
# baseline (speedup 1.0000x reference)
"""Trainium2 Bass kernel for the CAM-drop attention module.

Reference computation (per sample n):
    cams  = relu(W @ x[n])            # W: [C=64, Cin=1024], x[n]: [Cin, H*W]
    thr_k = gama * max_hw(cams[k])    # per-channel spatial max
    drop  = where(cams > thr, 0, cams)
    out[n] = x[n] * mean_k(drop)      # broadcast over Cin

Data-parallel over the batch: 32 samples sharded 4-per-core across 8
NeuronCores; fc_weights / gama replicated. No cross-core communication.

The problem is HBM-bound, so x is pre-cast to bf16 on the host and loaded
as bf16, and the output is stored as bf16 and widened to f32 on the host
(halves both HBM streams; rel err stays ~7e-3, well under the 2e-2 gate).
Matmuls accumulate bf16 into f32 PSUM; the channel mean is bf16.

Per-core pipeline (samples unrolled):
  - x[n] streamed as 8 bf16 tiles [128, 3136] into a rotating SBUF pool
  - cams accumulated in f32 PSUM over the 8 Cin tiles (7 banks of 448)
  - per-bank relu (ACT) + partial spatial max (DVE) chase the matmul tail
  - threshold, in-place drop-mask (DVE)
  - channel mean via a bf16 [64->128] ones/64 matmul (fused broadcast+scale),
    copied per-bank PSUM->SBUF on ACT
  - out tile (bf16) = xb tile * mean_bf16 (DVE 2x mode), DMA out; first
    product chunked per bank to chase the copies, its store on the scalar
    HWDGE ring; host widens the bf16 output back to f32
"""

import numpy as np

# Problem shape (hardcoded per harness contract).
N, CIN, H, W = 32, 1024, 56, 56
C = 64
HW = H * W          # 3136
NCORES = 8
NS = N // NCORES    # 4 samples per core
P = 128             # SBUF partitions
NT = CIN // P       # 8 Cin tiles
NCH = 7             # spatial chunks per sample
CH = HW // NCH      # 448 (fits one PSUM bank)
BANK = 512          # PSUM bank stride in f32 elements
NBBUF = 24          # rotating bf16 x-tile slots (0.784 MB each)
NOBUF = 7           # rotating bf16 out-tile slots (0.784 MB each)

_CACHE = {}


def _build_nc():
    from concourse import bacc, bass, tile
    from concourse import mybir

    f32 = mybir.dt.float32
    bf16 = mybir.dt.bfloat16
    alu = mybir.AluOpType

    nc = bacc.Bacc("TRN2", target_bir_lowering=False, debug=False)
    x_ext = nc.declare_dram_parameter("x", [NS, CIN, HW], bf16, isOutput=False)
    wT_ext = nc.declare_dram_parameter("fc_weights", [CIN, C], bf16, isOutput=False)
    g_ext = nc.declare_dram_parameter("gama", [C, 1], f32, isOutput=False)
    out_ext = nc.declare_dram_parameter("out", [NS, CIN, HW], bf16, isOutput=True)

    with tile.TileContext(nc) as tc:
        with (
            tc.tile_pool(name="consts", bufs=1) as constp,
            tc.tile_pool(name="xbp", bufs=NBBUF) as xbp,
            tc.tile_pool(name="outp", bufs=NOBUF) as outp,
            tc.tile_pool(name="stats", bufs=2) as statp,
            tc.tile_pool(name="camsb", bufs=1) as camp,
            tc.tile_pool(name="meanp", bufs=1) as meanp,
            tc.tile_pool(name="psum", bufs=1, space=bass.MemorySpace.PSUM) as psump,
        ):
            w_sb = constp.tile([P, NT, C], bf16)
            for t in range(NT):
                nc.sync.dma_start(out=w_sb[:, t, :], in_=wT_ext[t * P:(t + 1) * P, :])
            g_sb = constp.tile([C, 1], f32)
            nc.sync.dma_start(out=g_sb[:], in_=g_ext[:])
            ones_sb = constp.tile([C, P], bf16)
            nc.vector.memset(ones_sb[:], 1.0 / C)

            # PE clock warm-up: the HAM gate holds the PE at half clock until
            # ~4us of sustained matmul activity. Garbage matmuls into a spare
            # PSUM bank (never read; DCE keeps unread matmuls) warm it up
            # during the initial load-only DMA phase.
            warm_ps = psump.tile([C, BANK], f32, name="warm_ps", tag="warm")
            w_flat = w_sb[:].rearrange("p a b -> p (a b)")
            for _ in range(15):
                nc.tensor.matmul(
                    warm_ps[:, :], w_sb[:, 0, :], w_flat[:, 0:BANK],
                    start=True, stop=True,
                )

            for n in range(NS):
                xbs = []
                for t in range(NT):
                    xb = xbp.tile([P, HW], bf16, name=f"xb_{n}_{t}", tag="xb")
                    nc.sync.dma_start(out=xb[:], in_=x_ext[n, t * P:(t + 1) * P, :])
                    xbs.append(xb)

                cams = psump.tile([C, NCH, BANK], f32, name=f"cams_{n}", tag="ps")
                crelu = camp.tile([C, NCH, CH], bf16, name=f"crelu_{n}", tag="crelu")
                # All matmuls first, then the per-bank relus and partial
                # maxes: interleaving readers with the (t == NT-1) matmuls
                # makes Tile serialize each matmul behind the previous bank's
                # readers (tile-granularity WAR), stretching the tail ~4x.
                for t in range(NT):
                    for s in range(NCH):
                        nc.tensor.matmul(
                            cams[:, s, 0:CH],
                            w_sb[:, t, :],
                            xbs[t][:, s * CH:(s + 1) * CH],
                            start=(t == 0),
                            stop=(t == NT - 1),
                        )
                for s0, s1 in ((0, 4), (4, NCH)):
                    nc.scalar.activation(
                        crelu[:, s0:s1, :], cams[:, s0:s1, 0:CH],
                        mybir.ActivationFunctionType.Relu,
                    )
                # Spatial max from the SBUF relu copy (not PSUM) so the cams
                # banks' last reader is the relu — the PSUM slot turns over to
                # the mean matmuls sooner. max(crelu) == relu(max(cams)), so
                # thr = max(crelu) * gama directly.
                cmax = statp.tile([C, 1], f32, name=f"cmax_{n}", tag="cmax")
                nc.vector.tensor_reduce(
                    cmax[:], crelu[:, :, :], axis=mybir.AxisListType.XY,
                    op=alu.max,
                )
                thr = statp.tile([C, 1], f32, name=f"thr_{n}", tag="thr")
                nc.vector.tensor_scalar(
                    thr[:], cmax[:], g_sb[:], None, op0=alu.mult
                )

                # drop = crelu * (crelu <= thr), in place (comparing post-relu
                # values against thr >= 0 matches the reference's pre-relu
                # compare). Then the channel mean, broadcast to all 128
                # partitions via a ones/64 matmul. Emit all masks, then all
                # matmuls, then all PSUM->SBUF copies: interleaving them makes
                # Tile serialize each matmul behind the previous bank's copy
                # (WAR on the shared mean tile).
                mean_ps = psump.tile([P, NCH, BANK], f32, name=f"meanps_{n}", tag="ps")
                mean_sb = meanp.tile([P, HW], bf16, name=f"mean_{n}", tag="mean")
                mean_sb3 = mean_sb[:].rearrange("p (a b) -> p a b", a=NCH)
                for s0, s1 in ((0, 4), (4, NCH)):
                    nc.vector.scalar_tensor_tensor(
                        crelu[:, s0:s1, :], crelu[:, s0:s1, :], thr[:],
                        crelu[:, s0:s1, :], op0=alu.is_le, op1=alu.mult,
                    )
                for s in range(NCH):
                    nc.tensor.matmul(
                        mean_ps[:, s, 0:CH], ones_sb[:], crelu[:, s, :],
                        start=True, stop=True,
                    )
                for s in range(NCH):
                    nc.scalar.copy(mean_sb3[:, s, :], mean_ps[:, s, 0:CH])

                # First product is chunked per bank so it chases the ACT
                # copies instead of waiting for the full mean tile, and its
                # store goes out on the (idle) scalar HWDGE ring — both pull
                # the store stream start a few us earlier at each boundary.
                outs = [
                    outp.tile([P, HW], bf16, name=f"o_{n}_{t}", tag="ot")
                    for t in range(NT)
                ]
                o0 = outs[0][:].rearrange("p (a b) -> p a b", a=NCH)
                xb0 = xbs[0][:].rearrange("p (a b) -> p a b", a=NCH)
                for s in range(NCH):
                    nc.vector.tensor_mul(
                        o0[:, s, :], xb0[:, s, :], mean_sb3[:, s, :]
                    )
                nc.scalar.dma_start(out=out_ext[n, 0:P, :], in_=outs[0][:])
                for t in range(1, NT):
                    nc.vector.tensor_mul(outs[t][:], xbs[t][:], mean_sb[:])
                    nc.gpsimd.dma_start(
                        out=out_ext[n, t * P:(t + 1) * P, :], in_=outs[t][:]
                    )
    nc.compile()
    return nc


def _get_nc():
    if "nc" not in _CACHE:
        _CACHE["nc"] = _build_nc()
    return _CACHE["nc"]


def _make_in_maps(x, fc_weights, gama):
    from concourse import mybir

    bf16_np = mybir.dt.np(mybir.dt.bfloat16)
    x = np.asarray(x, dtype=np.float32)
    wT = np.ascontiguousarray(
        np.asarray(fc_weights, dtype=np.float32).reshape(C, CIN).T
    ).astype(bf16_np)
    g64 = np.ascontiguousarray(
        np.broadcast_to(np.asarray(gama, dtype=np.float32).reshape(1, 1), (C, 1))
    )
    return [
        {
            "x": np.ascontiguousarray(
                x[i * NS:(i + 1) * NS].reshape(NS, CIN, HW)
            ).astype(bf16_np),
            "fc_weights": wT,
            "gama": g64,
        }
        for i in range(NCORES)
    ]


def kernel(x: np.ndarray, fc_weights: np.ndarray, gama: np.ndarray) -> np.ndarray:
    from concourse.bass_utils import run_bass_kernel_spmd

    nc = _get_nc()
    in_maps = _make_in_maps(x, fc_weights, gama)
    res = run_bass_kernel_spmd(nc, in_maps, core_ids=list(range(NCORES)))
    out = np.concatenate(
        [
            res.results[i]["out"].astype(np.float32).reshape(NS, CIN, H, W)
            for i in range(NCORES)
        ],
        axis=0,
    )
    return out



# revision 2
# speedup vs baseline: 1.0733x; 1.0733x over previous
"""Trainium2 Bass kernel for the CAM-drop attention module.

Reference computation (per sample n):
    cams  = relu(W @ x[n])            # W: [C=64, Cin=1024], x[n]: [Cin, H*W]
    thr_k = gama * max_hw(cams[k])    # per-channel spatial max
    drop  = where(cams > thr, 0, cams)
    out[n] = x[n] * mean_k(drop)      # broadcast over Cin

Data-parallel over the batch: 32 samples sharded 4-per-core across 8
NeuronCores; fc_weights / gama replicated. No cross-core communication.

HBM-bound: x is pre-cast to bf16 on the host and loaded as bf16, the
output is stored as bf16 and widened to f32 on the host (halves both HBM
streams; rel err ~8e-3, well under the 2e-2 gate). 51.4 MB of traffic per
core at the ~420 GB/s sustained per-core DMA rate bounds exec at ~125 us.

Per-core pipeline, restructured for full cross-sample overlap (the v1
kernel serialized samples on a 7-bank PSUM WAR, making the per-sample
period ~30 us with HBM idle gaps):
  - PSUM is managed as 8 rotating single-bank tiles: per sample, 7 cams
    accumulator banks + 7 transient mean banks cycle through the pool, so
    the next sample's matmuls only wait on the fast per-bank relu/max
    readers instead of the whole mean-copy chain.
  - per-bank relu (ACT) and per-bank spatial max (DVE, straight from
    PSUM) chase the final accumulation matmuls; threshold compares in
    bf16 so the drop-mask runs at 2x DVE rate.
  - channel mean via a bf16 ones/64 matmul (fused broadcast+scale) one
    bank at a time, each copied PSUM->SBUF on ACT immediately.
  - out tiles (bf16) = xb * mean on DVE; muls+stores for sample n are
    emitted at the top of iteration n+1 so they fill engine idle time
    while the next sample's matmuls run. Stores split across the ACT
    HWDGE ring and the gpsimd SWDGE ring; x loads stream on the sync
    HWDGE ring; weights load via gpsimd so the sync ring starts on x
    immediately.
"""

import numpy as np

# Problem shape (hardcoded per harness contract).
N, CIN, H, W = 32, 1024, 56, 56
C = 64
HW = H * W          # 3136
NCORES = 8
NS = N // NCORES    # 4 samples per core
P = 128             # SBUF partitions
NT = CIN // P       # 8 Cin tiles
NCH = 7             # spatial chunks per sample (one PSUM bank each)
CH = HW // NCH      # 448
XBUF = 16           # rotating bf16 x-tile slots (2 samples)
OBUF = 10           # rotating bf16 out-tile slots

_CACHE = {}


def _build_nc():
    from concourse import bacc, bass, tile
    from concourse import mybir

    f32 = mybir.dt.float32
    bf16 = mybir.dt.bfloat16
    alu = mybir.AluOpType
    RELU = mybir.ActivationFunctionType.Relu
    AX = mybir.AxisListType.X

    nc = bacc.Bacc("TRN2", target_bir_lowering=False, debug=False)
    x_ext = nc.declare_dram_parameter("x", [NS, CIN, HW], bf16, isOutput=False)
    # fc_weights pre-swizzled on host to [P, NT*C]: element (p, t*C+c) =
    # W[c, t*P+p], so w_sb[:, t*C:(t+1)*C] is the lhsT for Cin-tile t.
    w_ext = nc.declare_dram_parameter("fc_weights", [P, NT * C], bf16, isOutput=False)
    g_ext = nc.declare_dram_parameter("gama", [C, 1], f32, isOutput=False)
    out_ext = nc.declare_dram_parameter("out", [NS, CIN, HW], bf16, isOutput=True)

    with tile.TileContext(nc) as tc:
        with (
            tc.tile_pool(name="consts", bufs=1) as constp,
            tc.tile_pool(name="xbp", bufs=XBUF) as xbp,
            tc.tile_pool(name="outp", bufs=OBUF) as outp,
            tc.tile_pool(name="crelup", bufs=2) as crelup,
            tc.tile_pool(name="meanp", bufs=2) as meanp,
            tc.tile_pool(name="statp", bufs=2) as statp,
            tc.tile_pool(name="psump", bufs=8, space=bass.MemorySpace.PSUM) as psump,
        ):
            # Consts off the sync ring (gpsimd SWDGE) so x loads start at once.
            w_sb = constp.tile([P, NT * C], bf16)
            nc.gpsimd.dma_start(out=w_sb[:], in_=w_ext[:])
            g_sb = constp.tile([C, 1], f32)
            nc.gpsimd.dma_start(out=g_sb[:], in_=g_ext[:])
            ones_sb = constp.tile([C, P], bf16)
            nc.vector.memset(ones_sb[:], 1.0 / C)

            # PE clock warm-up: the HAM gate holds the PE at half clock until
            # ~4us of sustained matmul activity. Garbage matmuls into a spare
            # rotating bank during the initial load-only DMA phase.
            warm = psump.tile([C, CH], f32, name="warm", tag="bank")
            for _ in range(15):
                nc.tensor.matmul(
                    warm[:, :], w_sb[:, 0:C], w_sb[:, 0:CH], start=True, stop=True
                )

            state = {}

            def emit_muls_stores(m):
                xbs_m, mean_m = state.pop(m)
                outs = [
                    outp.tile([P, HW], bf16, name=f"o_{m}_{t}", tag="ot")
                    for t in range(NT)
                ]
                for t in range(NT):
                    nc.vector.tensor_mul(outs[t][:], xbs_m[t][:], mean_m[:])
                    if t < 4:
                        nc.scalar.dma_start(
                            out=out_ext[m, t * P:(t + 1) * P, :], in_=outs[t][:]
                        )
                    else:
                        nc.gpsimd.dma_start(
                            out=out_ext[m, t * P:(t + 1) * P, :], in_=outs[t][:]
                        )

            for n in range(NS):
                xbs = []
                for t in range(NT):
                    xb = xbp.tile([P, HW], bf16, name=f"xb_{n}_{t}", tag="xb")
                    nc.sync.dma_start(out=xb[:], in_=x_ext[n, t * P:(t + 1) * P, :])
                    xbs.append(xb)

                # Previous sample's element-wise muls + stores run while this
                # sample's matmuls accumulate.
                if n > 0:
                    emit_muls_stores(n - 1)

                cams = [
                    psump.tile([C, CH], f32, name=f"c_{n}_{s}", tag="bank")
                    for s in range(NCH)
                ]
                for t in range(NT):
                    for s in range(NCH):
                        nc.tensor.matmul(
                            cams[s][:, :],
                            w_sb[:, t * C:(t + 1) * C],
                            xbs[t][:, s * CH:(s + 1) * CH],
                            start=(t == 0),
                            stop=(t == NT - 1),
                        )

                # Per-bank relu + spatial max chase the final accumulation
                # matmuls; each bank's last PSUM readers finish ~1us after its
                # stop matmul, freeing it for the mean matmuls / next sample.
                crelu = crelup.tile([C, NCH, CH], bf16, name=f"crelu_{n}", tag="cr")
                pm = statp.tile([C, NCH], f32, name=f"pm_{n}", tag="pm")
                for s in range(NCH):
                    nc.scalar.activation(crelu[:, s, :], cams[s][:, :], RELU)
                    nc.vector.tensor_reduce(
                        pm[:, s:s + 1], cams[s][:, :], axis=AX, op=alu.max
                    )

                # thr = gama * max(cams); pre-relu max is safe: it differs only
                # when every cam in the channel is negative, and then crelu==0
                # so the masked product is 0 either way. bf16 thr keeps the
                # drop-mask compare all-16-bit (2x DVE rate).
                cmax = statp.tile([C, 1], f32, name=f"cmax_{n}", tag="cmax")
                nc.vector.tensor_reduce(cmax[:], pm[:, :], axis=AX, op=alu.max)
                thr = statp.tile([C, 1], bf16, name=f"thr_{n}", tag="thr")
                nc.vector.tensor_scalar(thr[:], cmax[:], g_sb[:], None, op0=alu.mult)

                # drop = crelu * (crelu <= thr), in place (comparing post-relu
                # values against thr matches the reference's pre-relu compare).
                for s0, s1 in ((0, 4), (4, NCH)):
                    nc.vector.scalar_tensor_tensor(
                        crelu[:, s0:s1, :], crelu[:, s0:s1, :], thr[:],
                        crelu[:, s0:s1, :], op0=alu.is_le, op1=alu.mult,
                    )

                # Channel mean via ones/64 matmul, one rotating PSUM bank per
                # chunk; the bank each mean matmul lands on was freed by a
                # per-bank relu/max long before, so the matmuls run
                # back-to-back with the ACT copies chasing.
                mean_sb = meanp.tile([P, HW], bf16, name=f"mean_{n}", tag="mean")
                m3 = mean_sb[:].rearrange("p (a b) -> p a b", a=NCH)
                for s in range(NCH):
                    mps = psump.tile([P, CH], f32, name=f"m_{n}_{s}", tag="bank")
                    nc.tensor.matmul(
                        mps[:, :], ones_sb[:], crelu[:, s, :], start=True, stop=True
                    )
                    nc.scalar.copy(m3[:, s, :], mps[:, :])

                state[n] = (xbs, mean_sb)

            emit_muls_stores(NS - 1)
    nc.compile()
    return nc


def _get_nc():
    if "nc" not in _CACHE:
        _CACHE["nc"] = _build_nc()
    return _CACHE["nc"]


def _make_in_maps(x, fc_weights, gama):
    from concourse import mybir

    bf16_np = mybir.dt.np(mybir.dt.bfloat16)
    x = np.asarray(x, dtype=np.float32)
    # [CIN, C] -> [NT, P, C] -> [P, NT, C] -> [P, NT*C]
    wL = np.ascontiguousarray(
        np.asarray(fc_weights, dtype=np.float32)
        .reshape(C, CIN).T
        .reshape(NT, P, C)
        .transpose(1, 0, 2)
        .reshape(P, NT * C)
    ).astype(bf16_np)
    g64 = np.ascontiguousarray(
        np.broadcast_to(np.asarray(gama, dtype=np.float32).reshape(1, 1), (C, 1))
    )
    return [
        {
            "x": np.ascontiguousarray(
                x[i * NS:(i + 1) * NS].reshape(NS, CIN, HW)
            ).astype(bf16_np),
            "fc_weights": wL,
            "gama": g64,
        }
        for i in range(NCORES)
    ]


def kernel(x: np.ndarray, fc_weights: np.ndarray, gama: np.ndarray) -> np.ndarray:
    from concourse.bass_utils import run_bass_kernel_spmd

    nc = _get_nc()
    in_maps = _make_in_maps(x, fc_weights, gama)
    res = run_bass_kernel_spmd(nc, in_maps, core_ids=list(range(NCORES)))
    out = np.concatenate(
        [
            res.results[i]["out"].astype(np.float32).reshape(NS, CIN, H, W)
            for i in range(NCORES)
        ],
        axis=0,
    )
    return out
